# revision 9
# baseline (speedup 1.0000x reference)
"""AdaptiveDiffusionLayer on 8 TRN2 NeuronCores.

out = (1 - t) * support + t * (adj @ support),  support = x @ weight

Strategy (1D column-parallel SpMM + chunked ReduceScatter):
  - Column-shard adj across 8 cores: core c holds adj[:, c*1250:(c+1)*1250],
    pre-transposed + bf16-cast on the host so the contraction index k lands
    on the SBUF partition axis with unit-stride DMA. Shard x by the SAME k
    rows, so each core's needed support block (support_c = x_c @ W) is
    entirely LOCAL — no all-gather on the critical path.
  - Each core computes partial[i, :] = adj[i, own k] @ support_c for ALL
    10000 output rows, in 10 batches of 8 PSUM accumulators (125-row
    i-tiles, 10 k-tiles of 125 each). Partials are cast to bf16 and
    reduce-scattered in 5 pipelined chunks (the adjT columns are permuted
    host-side so each chunk's rank-r slice is exactly rank r's own output
    rows). Compute of chunk g+1 overlaps the ReduceScatter of chunk g, and
    multi-core launch skew is absorbed by compute instead of idling the PE.
  - Fused epilogue per chunk: out = t * rs_sum + (1-t) * support_c.
"""

import sys

for _p in ("/opt/trn_rl_repo",):
    if _p not in sys.path:
        sys.path.append(_p)

import numpy as np
import ml_dtypes

from concourse import bass, bacc, mybir, tile
from concourse.bass_utils import run_bass_kernel_spmd

N = 10000
IN_F = 512
OUT_F = 512
C = 8               # cores
R = N // C          # 1250 k-rows (adj columns / x rows) per core
SUB = 125           # i-tile rows / k-tile rows
NQ = R // SUB       # 10 local k-tiles
NB = 8              # PSUM accumulators (i-tiles) per batch
BATCH = NB * SUB    # 1000 output rows per batch
NBATCH = N // BATCH  # 10 batches
G = 5               # ReduceScatter chunks (2 batches each)
CHROW = 2 * BATCH   # 2000 rows per chunk

BF16 = mybir.dt.bfloat16
F32 = mybir.dt.float32

_cached = {}


def _build():
    nc = bacc.Bacc("TRN2", target_bir_lowering=False, debug=False, num_devices=C)

    adjt = nc.dram_tensor("adjt", [R, N], BF16, kind="ExternalInput")
    xt = nc.dram_tensor("xt", [IN_F, R], BF16, kind="ExternalInput")
    w = nc.dram_tensor("w", [IN_F, OUT_F], BF16, kind="ExternalInput")
    tsc = nc.dram_tensor("tsc", [128, 2], F32, kind="ExternalInput")
    out = nc.dram_tensor("out", [R, OUT_F], F32, kind="ExternalOutput")

    rs_in = [
        nc.dram_tensor(f"rs_in{g}", [CHROW, OUT_F], BF16) for g in range(G)
    ]
    rs_out = [
        nc.dram_tensor(f"rs_out{g}", [CHROW // C, OUT_F], BF16) for g in range(G)
    ]

    NJ = IN_F // 128  # 4 contraction tiles for x @ W

    with tile.TileContext(nc) as tc:
        with (
            tc.tile_pool(name="persist", bufs=1) as p_pers,
            tc.tile_pool(name="supbf_pool", bufs=1) as p_supbf,
            tc.tile_pool(name="slab_pool", bufs=16) as p_slab,
            tc.tile_pool(name="part_pool", bufs=8) as p_part,
            tc.tile_pool(name="ep_pool", bufs=4) as p_ep,
        ):
            xt_sb = p_pers.tile([128, NJ * R], BF16, tag="xt_sb", name="xt_sb")
            w_sb = p_pers.tile([128, NJ * OUT_F], BF16, tag="w_sb", name="w_sb")
            tsc_sb = p_pers.tile([128, 2], F32, tag="tsc_sb", name="tsc_sb")

            for j in range(NJ):
                nc.scalar.dma_start(
                    out=xt_sb[:, j * R:(j + 1) * R],
                    in_=xt[j * 128:(j + 1) * 128, :],
                )
                nc.scalar.dma_start(
                    out=w_sb[:, j * OUT_F:(j + 1) * OUT_F],
                    in_=w[j * 128:(j + 1) * 128, :],
                )
            nc.scalar.dma_start(out=tsc_sb[:, :], in_=tsc[:, :])

            # ---- support_c = x_c @ W, 10 k-subtiles of 125 rows (all local) ----
            supbf = []
            with tc.tile_pool(name="psum_sup", bufs=3, space="PSUM") as pp_sup:
                for s in range(NQ):
                    ps = pp_sup.tile([SUB, OUT_F], F32, tag="ps", name=f"ps{s}")
                    for j in range(NJ):
                        nc.tensor.matmul(
                            ps[:, :],
                            lhsT=xt_sb[:, j * R + s * SUB: j * R + (s + 1) * SUB],
                            rhs=w_sb[:, j * OUT_F:(j + 1) * OUT_F],
                            start=(j == 0),
                            stop=(j == NJ - 1),
                        )
                    sb = p_supbf.tile(
                        [SUB, OUT_F], BF16, tag=f"supbf{s}", name=f"supbf{s}"
                    )
                    nc.vector.tensor_copy(sb[:, :], ps[:, :])
                    supbf.append(sb)

            # ---- main SpMM: 10 batches x 8 PSUM accumulators over 10 k-tiles;
            # every 2 batches feed one pipelined ReduceScatter chunk ----
            with tc.tile_pool(name="psum_main", bufs=1, space="PSUM") as pp_main:
                for b in range(NBATCH):
                    g, h = divmod(b, 2)
                    acc = [
                        pp_main.tile(
                            [SUB, OUT_F], F32, tag=f"acc{it}", name=f"acc{b}_{it}"
                        )
                        for it in range(NB)
                    ]
                    for q in range(NQ):
                        slab = p_slab.tile(
                            [SUB, BATCH], BF16, tag="slab", name=f"slab{b}_{q}"
                        )
                        nc.sync.dma_start(
                            out=slab[:, :],
                            in_=adjt[q * SUB:(q + 1) * SUB,
                                     b * BATCH:(b + 1) * BATCH],
                        )
                        for it in range(NB):
                            nc.tensor.matmul(
                                acc[it][:, :],
                                lhsT=slab[:, it * SUB:(it + 1) * SUB],
                                rhs=supbf[q][:, :],
                                start=(q == 0),
                                stop=(q == NQ - 1),
                            )
                    for it in range(NB):
                        pt = p_part.tile(
                            [SUB, OUT_F], BF16, tag="pt", name=f"pt{b}_{it}"
                        )
                        nc.vector.tensor_copy(pt[:, :], acc[it][:, :])
                        nc.scalar.dma_start(
                            out=rs_in[g][h * BATCH + it * SUB:
                                         h * BATCH + (it + 1) * SUB, :],
                            in_=pt[:, :],
                        )
                    if h == 1:
                        nc.gpsimd.collective_compute(
                            "ReduceScatter",
                            mybir.AluOpType.add,
                            replica_groups=[list(range(C))],
                            ins=[rs_in[g].ap().opt()],
                            outs=[rs_out[g].ap().opt()],
                        )
                        # epilogue for this chunk's two 125-row subtiles
                        for h2 in range(2):
                            s = 2 * g + h2
                            rs_sb = p_ep.tile(
                                [SUB, OUT_F], BF16, tag="rs_sb", name=f"rs{s}"
                            )
                            nc.scalar.dma_start(
                                out=rs_sb[:, :],
                                in_=rs_out[g][h2 * SUB:(h2 + 1) * SUB, :],
                            )
                            sc = p_ep.tile(
                                [SUB, OUT_F], F32, tag="sc", name=f"osc{s}"
                            )
                            nc.vector.tensor_scalar_mul(
                                sc[:, :], supbf[s][:, :], tsc_sb[0:SUB, 1:2]
                            )
                            ot = p_ep.tile(
                                [SUB, OUT_F], F32, tag="ot", name=f"ot{s}"
                            )
                            nc.vector.scalar_tensor_tensor(
                                ot[:, :],
                                rs_sb[:, :],
                                tsc_sb[0:SUB, 0:1],
                                sc[:, :],
                                mybir.AluOpType.mult,
                                mybir.AluOpType.add,
                            )
                            nc.scalar.dma_start(
                                out=out[s * SUB:(s + 1) * SUB, :], in_=ot[:, :]
                            )

    nc.compile()
    return nc


def _i_perm():
    """Output-row permutation matching the chunked ReduceScatter layout:
    chunk g covers rows [r*R + g*CH, r*R + (g+1)*CH) for each rank r, in
    rank order, so each chunk's rank-r slice is rank r's own rows."""
    CH = CHROW // C  # 250 rows per rank per chunk
    perm = np.empty(N, np.int64)
    pos = 0
    for g in range(G):
        for r in range(C):
            base = r * R + g * CH
            perm[pos:pos + CH] = np.arange(base, base + CH)
            pos += CH
    return perm


def _shard_inputs(x, adj, t, weight):
    bf16 = ml_dtypes.bfloat16
    w_bf = np.asarray(weight, np.float32).astype(bf16)
    t0 = float(np.asarray(t, np.float32).reshape(-1)[0])
    tsc = np.empty((128, 2), np.float32)
    tsc[:, 0] = t0
    tsc[:, 1] = 1.0 - t0

    x = np.asarray(x, np.float32)
    adj = np.asarray(adj, np.float32)
    perm = _i_perm()
    adj_p = adj[perm]  # permuted output rows

    in_maps = []
    for c in range(C):
        cols = slice(c * R, (c + 1) * R)
        adjt = np.ascontiguousarray(adj_p[:, cols].T).astype(bf16)  # [R, N]
        xt = np.ascontiguousarray(x[cols].T).astype(bf16)           # [IN_F, R]
        in_maps.append({"adjt": adjt, "xt": xt, "w": w_bf, "tsc": tsc})
    return in_maps


def kernel(x, adj, t, weight):
    if "nc" not in _cached:
        _cached["nc"] = _build()
    nc = _cached["nc"]
    in_maps = _shard_inputs(x, adj, t, weight)
    res = run_bass_kernel_spmd(nc, in_maps, list(range(C)))
    return np.concatenate([res.results[c]["out"] for c in range(C)], axis=0)


# revision 12
# speedup vs baseline: 1.0049x; 1.0049x over previous
"""AdaptiveDiffusionLayer on 8 TRN2 NeuronCores.

out = (1 - t) * support + t * (adj @ support),  support = x @ weight

Strategy (1D column-parallel SpMM + chunked ReduceScatter):
  - Column-shard adj across 8 cores: core c holds adj[:, c*1250:(c+1)*1250],
    pre-transposed + bf16-cast on the host so the contraction index k lands
    on the SBUF partition axis with unit-stride DMA. Shard x by the SAME k
    rows, so each core's needed support block (support_c = x_c @ W) is
    entirely LOCAL — no all-gather on the critical path.
  - Each core computes partial[i, :] = adj[i, own k] @ support_c for ALL
    10000 output rows, in 10 batches of 8 PSUM accumulators (125-row
    i-tiles, 10 k-tiles of 125 each). Partials are cast to bf16 and
    reduce-scattered in 5 pipelined chunks (the adjT columns are permuted
    host-side so each chunk's rank-r slice is exactly rank r's own output
    rows). Compute of chunk g+1 overlaps the ReduceScatter of chunk g, and
    multi-core launch skew is absorbed by compute instead of idling the PE.
  - Fused epilogue per chunk: out = t * rs_sum + (1-t) * support_c.
"""

import sys

for _p in ("/opt/trn_rl_repo",):
    if _p not in sys.path:
        sys.path.append(_p)

import numpy as np
import ml_dtypes

from concourse import bass, bacc, mybir, tile
from concourse.bass_utils import run_bass_kernel_spmd

N = 10000
IN_F = 512
OUT_F = 512
C = 8               # cores
R = N // C          # 1250 k-rows (adj columns / x rows) per core
SUB = 125           # i-tile rows / k-tile rows
NQ = R // SUB       # 10 local k-tiles
NB = 8              # PSUM accumulators (i-tiles) per batch
BATCH = NB * SUB    # 1000 output rows per batch
NBATCH = N // BATCH  # 10 batches
G = 5               # ReduceScatter chunks (2 batches each)
CHROW = 2 * BATCH   # 2000 rows per chunk

BF16 = mybir.dt.bfloat16
F32 = mybir.dt.float32

_cached = {}


def _build():
    nc = bacc.Bacc("TRN2", target_bir_lowering=False, debug=False, num_devices=C)

    adjt = nc.dram_tensor("adjt", [R, N], BF16, kind="ExternalInput")
    xt = nc.dram_tensor("xt", [IN_F, R], BF16, kind="ExternalInput")
    w = nc.dram_tensor("w", [IN_F, OUT_F], BF16, kind="ExternalInput")
    tsc = nc.dram_tensor("tsc", [128, 2], F32, kind="ExternalInput")
    out = nc.dram_tensor("out", [R, OUT_F], F32, kind="ExternalOutput")

    rs_in = [
        nc.dram_tensor(f"rs_in{g}", [CHROW, OUT_F], BF16) for g in range(G)
    ]
    rs_out = [
        nc.dram_tensor(f"rs_out{g}", [CHROW // C, OUT_F], BF16) for g in range(G)
    ]

    NJ = IN_F // 128  # 4 contraction tiles for x @ W

    with tile.TileContext(nc) as tc:
        with (
            tc.tile_pool(name="persist", bufs=1) as p_pers,
            tc.tile_pool(name="supbf_pool", bufs=1) as p_supbf,
            tc.tile_pool(name="slab_pool", bufs=24) as p_slab,
            tc.tile_pool(name="part_pool", bufs=16) as p_part,
            tc.tile_pool(name="ep_pool", bufs=4) as p_ep,
        ):
            xt_sb = p_pers.tile([128, NJ * R], BF16, tag="xt_sb", name="xt_sb")
            w_sb = p_pers.tile([128, NJ * OUT_F], BF16, tag="w_sb", name="w_sb")
            tsc_sb = p_pers.tile([128, 2], F32, tag="tsc_sb", name="tsc_sb")

            for j in range(NJ):
                nc.scalar.dma_start(
                    out=xt_sb[:, j * R:(j + 1) * R],
                    in_=xt[j * 128:(j + 1) * 128, :],
                )
                nc.scalar.dma_start(
                    out=w_sb[:, j * OUT_F:(j + 1) * OUT_F],
                    in_=w[j * 128:(j + 1) * 128, :],
                )
            nc.scalar.dma_start(out=tsc_sb[:, :], in_=tsc[:, :])

            # ---- support_c = x_c @ W, 10 k-subtiles of 125 rows (all local) ----
            supbf = []
            supsc = []
            with tc.tile_pool(name="psum_sup", bufs=3, space="PSUM") as pp_sup:
                for s in range(NQ):
                    ps = pp_sup.tile([SUB, OUT_F], F32, tag="ps", name=f"ps{s}")
                    for j in range(NJ):
                        nc.tensor.matmul(
                            ps[:, :],
                            lhsT=xt_sb[:, j * R + s * SUB: j * R + (s + 1) * SUB],
                            rhs=w_sb[:, j * OUT_F:(j + 1) * OUT_F],
                            start=(j == 0),
                            stop=(j == NJ - 1),
                        )
                    sb = p_supbf.tile(
                        [SUB, OUT_F], BF16, tag=f"supbf{s}", name=f"supbf{s}"
                    )
                    nc.vector.tensor_copy(sb[:, :], ps[:, :])
                    supbf.append(sb)
                    sc = p_supbf.tile(
                        [SUB, OUT_F], F32, tag=f"supsc{s}", name=f"supsc{s}"
                    )
                    nc.vector.tensor_scalar_mul(
                        sc[:, :], ps[:, :], tsc_sb[0:SUB, 1:2]
                    )
                    supsc.append(sc)

            # ---- main SpMM: 10 batches x 8 PSUM accumulators over 10 k-tiles;
            # every 2 batches feed one pipelined ReduceScatter chunk. Each
            # chunk's epilogue is emitted two chunks later in program order
            # so its RS-completion wait is already satisfied when the
            # in-order vector engine reaches it (no pipeline stall). ----
            def epilogue(g):
                for h2 in range(2):
                    s = 2 * g + h2
                    rs_sb = p_ep.tile(
                        [SUB, OUT_F], BF16, tag="rs_sb", name=f"rs{s}"
                    )
                    nc.scalar.dma_start(
                        out=rs_sb[:, :],
                        in_=rs_out[g][h2 * SUB:(h2 + 1) * SUB, :],
                    )
                    ot = p_ep.tile([SUB, OUT_F], F32, tag="ot", name=f"ot{s}")
                    nc.vector.scalar_tensor_tensor(
                        ot[:, :],
                        rs_sb[:, :],
                        tsc_sb[0:SUB, 0:1],
                        supsc[s][:, :],
                        mybir.AluOpType.mult,
                        mybir.AluOpType.add,
                    )
                    nc.scalar.dma_start(
                        out=out[s * SUB:(s + 1) * SUB, :], in_=ot[:, :]
                    )

            with tc.tile_pool(name="psum_main", bufs=1, space="PSUM") as pp_main:
                for b in range(NBATCH):
                    g, h = divmod(b, 2)
                    acc = [
                        pp_main.tile(
                            [SUB, OUT_F], F32, tag=f"acc{it}", name=f"acc{b}_{it}"
                        )
                        for it in range(NB)
                    ]
                    for q in range(NQ):
                        slab = p_slab.tile(
                            [SUB, BATCH], BF16, tag="slab", name=f"slab{b}_{q}"
                        )
                        nc.sync.dma_start(
                            out=slab[:, :],
                            in_=adjt[q * SUB:(q + 1) * SUB,
                                     b * BATCH:(b + 1) * BATCH],
                        )
                        for it in range(NB):
                            nc.tensor.matmul(
                                acc[it][:, :],
                                lhsT=slab[:, it * SUB:(it + 1) * SUB],
                                rhs=supbf[q][:, :],
                                start=(q == 0),
                                stop=(q == NQ - 1),
                            )
                    for it in range(NB):
                        pt = p_part.tile(
                            [SUB, OUT_F], BF16, tag="pt", name=f"pt{b}_{it}"
                        )
                        nc.vector.tensor_copy(pt[:, :], acc[it][:, :])
                        nc.scalar.dma_start(
                            out=rs_in[g][h * BATCH + it * SUB:
                                         h * BATCH + (it + 1) * SUB, :],
                            in_=pt[:, :],
                        )
                    if h == 1:
                        nc.gpsimd.collective_compute(
                            "ReduceScatter",
                            mybir.AluOpType.add,
                            replica_groups=[list(range(C))],
                            ins=[rs_in[g].ap().opt()],
                            outs=[rs_out[g].ap().opt()],
                        )
                    if b >= 5 and b % 2 == 1:
                        epilogue((b - 5) // 2)
                for g in range(G - 2, G):
                    epilogue(g)

    nc.compile()
    return nc


def _i_perm():
    """Output-row permutation matching the chunked ReduceScatter layout:
    chunk g covers rows [r*R + g*CH, r*R + (g+1)*CH) for each rank r, in
    rank order, so each chunk's rank-r slice is rank r's own rows."""
    CH = CHROW // C  # 250 rows per rank per chunk
    perm = np.empty(N, np.int64)
    pos = 0
    for g in range(G):
        for r in range(C):
            base = r * R + g * CH
            perm[pos:pos + CH] = np.arange(base, base + CH)
            pos += CH
    return perm


def _shard_inputs(x, adj, t, weight):
    bf16 = ml_dtypes.bfloat16
    w_bf = np.asarray(weight, np.float32).astype(bf16)
    t0 = float(np.asarray(t, np.float32).reshape(-1)[0])
    tsc = np.empty((128, 2), np.float32)
    tsc[:, 0] = t0
    tsc[:, 1] = 1.0 - t0

    x = np.asarray(x, np.float32)
    adj = np.asarray(adj, np.float32)
    perm = _i_perm()
    adj_p = adj[perm]  # permuted output rows

    in_maps = []
    for c in range(C):
        cols = slice(c * R, (c + 1) * R)
        adjt = np.ascontiguousarray(adj_p[:, cols].T).astype(bf16)  # [R, N]
        xt = np.ascontiguousarray(x[cols].T).astype(bf16)           # [IN_F, R]
        in_maps.append({"adjt": adjt, "xt": xt, "w": w_bf, "tsc": tsc})
    return in_maps


def kernel(x, adj, t, weight):
    if "nc" not in _cached:
        _cached["nc"] = _build()
    nc = _cached["nc"]
    in_maps = _shard_inputs(x, adj, t, weight)
    res = run_bass_kernel_spmd(nc, in_maps, list(range(C)))
    return np.concatenate([res.results[c]["out"] for c in range(C)], axis=0)


# revision 14
# speedup vs baseline: 1.1084x; 1.1030x over previous
"""AdaptiveDiffusionLayer on 8 TRN2 NeuronCores.

out = (1 - t) * support + t * (adj @ support),  support = x @ weight

Strategy (1D column-parallel SpMM + chunked ReduceScatter):
  - Column-shard adj across 8 cores: core c holds adj[:, c*1250:(c+1)*1250],
    pre-transposed + bf16-cast on the host so the contraction index k lands
    on the SBUF partition axis with unit-stride DMA. Shard x by the SAME k
    rows, so each core's needed support block (support_c = x_c @ W) is
    entirely LOCAL — no all-gather on the critical path.
  - Each core computes partial[i, :] = adj[i, own k] @ support_c for ALL
    10000 output rows, in 10 batches of 8 PSUM accumulators (125-row
    i-tiles, 10 k-tiles of 125 each). Partials are cast to bf16 and
    reduce-scattered in 5 pipelined chunks (the adjT columns are permuted
    host-side so each chunk's rank-r slice is exactly rank r's own output
    rows). Compute of chunk g+1 overlaps the ReduceScatter of chunk g, and
    multi-core launch skew is absorbed by compute instead of idling the PE.
  - Fused epilogue per chunk: out = t * rs_sum + (1-t) * support_c.
"""

import sys

for _p in ("/opt/trn_rl_repo",):
    if _p not in sys.path:
        sys.path.append(_p)

import numpy as np
import ml_dtypes

from concourse import bass, bacc, mybir, tile
from concourse.bass_utils import run_bass_kernel_spmd

N = 10000
IN_F = 512
OUT_F = 512
C = 8               # cores
R = N // C          # 1250 k-rows (adj columns / x rows) per core
SUB = 125           # i-tile rows / k-tile rows
NQ = R // SUB       # 10 local k-tiles
NB = 8              # PSUM accumulators (i-tiles) per batch
BATCH = NB * SUB    # 1000 output rows per batch
NBATCH = N // BATCH  # 10 batches
CHUNK_BATCHES = [2, 2, 2, 2, 1, 1]   # batches per ReduceScatter chunk
G = len(CHUNK_BATCHES)
CHUNK_START = [sum(CHUNK_BATCHES[:g]) for g in range(G)]  # first batch of chunk
BATCH_CHUNK = []                       # batch -> (chunk, index within chunk)
for _g, _n in enumerate(CHUNK_BATCHES):
    for _h in range(_n):
        BATCH_CHUNK.append((_g, _h))

BF16 = mybir.dt.bfloat16
F32 = mybir.dt.float32

_cached = {}


def _build():
    nc = bacc.Bacc("TRN2", target_bir_lowering=False, debug=False, num_devices=C)

    adjt = nc.dram_tensor("adjt", [R, N], BF16, kind="ExternalInput")
    xt = nc.dram_tensor("xt", [IN_F, R], BF16, kind="ExternalInput")
    w = nc.dram_tensor("w", [IN_F, OUT_F], BF16, kind="ExternalInput")
    tsc = nc.dram_tensor("tsc", [128, 2], F32, kind="ExternalInput")
    out = nc.dram_tensor("out", [R, OUT_F], F32, kind="ExternalOutput")

    rs_in = [
        nc.dram_tensor(f"rs_in{g}", [CHUNK_BATCHES[g] * BATCH, OUT_F], BF16)
        for g in range(G)
    ]
    rs_out = [
        nc.dram_tensor(f"rs_out{g}", [CHUNK_BATCHES[g] * BATCH // C, OUT_F], BF16)
        for g in range(G)
    ]

    NJ = IN_F // 128  # 4 contraction tiles for x @ W

    with tile.TileContext(nc) as tc:
        with (
            tc.tile_pool(name="persist", bufs=1) as p_pers,
            tc.tile_pool(name="supbf_pool", bufs=1) as p_supbf,
            tc.tile_pool(name="slab_pool", bufs=12) as p_slab,
            tc.tile_pool(name="part_pool", bufs=16) as p_part,
            tc.tile_pool(name="ep_pool", bufs=4) as p_ep,
        ):
            xt_sb = p_pers.tile([128, NJ * R], BF16, tag="xt_sb", name="xt_sb")
            w_sb = p_pers.tile([128, NJ * OUT_F], BF16, tag="w_sb", name="w_sb")
            tsc_sb = p_pers.tile([128, 2], F32, tag="tsc_sb", name="tsc_sb")

            for j in range(NJ):
                nc.scalar.dma_start(
                    out=xt_sb[:, j * R:(j + 1) * R],
                    in_=xt[j * 128:(j + 1) * 128, :],
                )
                nc.scalar.dma_start(
                    out=w_sb[:, j * OUT_F:(j + 1) * OUT_F],
                    in_=w[j * 128:(j + 1) * 128, :],
                )
            nc.scalar.dma_start(out=tsc_sb[:, :], in_=tsc[:, :])

            # ---- support_c = x_c @ W, 10 k-subtiles of 125 rows (all local) ----
            supbf = []
            supsc = []
            with tc.tile_pool(name="psum_sup", bufs=3, space="PSUM") as pp_sup:
                for s in range(NQ):
                    ps = pp_sup.tile([SUB, OUT_F], F32, tag="ps", name=f"ps{s}")
                    for j in range(NJ):
                        nc.tensor.matmul(
                            ps[:, :],
                            lhsT=xt_sb[:, j * R + s * SUB: j * R + (s + 1) * SUB],
                            rhs=w_sb[:, j * OUT_F:(j + 1) * OUT_F],
                            start=(j == 0),
                            stop=(j == NJ - 1),
                        )
                    sb = p_supbf.tile(
                        [SUB, OUT_F], BF16, tag=f"supbf{s}", name=f"supbf{s}"
                    )
                    nc.vector.tensor_copy(sb[:, :], ps[:, :])
                    supbf.append(sb)
                    sc = p_supbf.tile(
                        [SUB, OUT_F], F32, tag=f"supsc{s}", name=f"supsc{s}"
                    )
                    nc.vector.tensor_scalar_mul(
                        sc[:, :], ps[:, :], tsc_sb[0:SUB, 1:2]
                    )
                    supsc.append(sc)

            # ---- main SpMM: 10 batches x 8 PSUM accumulators over 10 k-tiles;
            # every 2 batches feed one pipelined ReduceScatter chunk. Each
            # chunk's epilogue is emitted two chunks later in program order
            # so its RS-completion wait is already satisfied when the
            # in-order vector engine reaches it (no pipeline stall). ----
            def epilogue(g):
                for h2 in range(CHUNK_BATCHES[g]):
                    s = CHUNK_START[g] + h2
                    rs_sb = p_ep.tile(
                        [SUB, OUT_F], BF16, tag="rs_sb", name=f"rs{s}"
                    )
                    nc.gpsimd.dma_start(
                        out=rs_sb[:, :],
                        in_=rs_out[g][h2 * SUB:(h2 + 1) * SUB, :],
                    )
                    ot = p_ep.tile([SUB, OUT_F], F32, tag="ot", name=f"ot{s}")
                    nc.vector.scalar_tensor_tensor(
                        ot[:, :],
                        rs_sb[:, :],
                        tsc_sb[0:SUB, 0:1],
                        supsc[s][:, :],
                        mybir.AluOpType.mult,
                        mybir.AluOpType.add,
                    )
                    nc.gpsimd.dma_start(
                        out=out[s * SUB:(s + 1) * SUB, :], in_=ot[:, :]
                    )

            with tc.tile_pool(name="psum_main", bufs=1, space="PSUM") as pp_main:
                ep_next = 0
                for b in range(NBATCH):
                    g, h = BATCH_CHUNK[b]
                    acc = [
                        pp_main.tile(
                            [SUB, OUT_F], F32, tag=f"acc{it}", name=f"acc{b}_{it}"
                        )
                        for it in range(NB)
                    ]
                    for qp in range(NQ // 2):
                        slab = p_slab.tile(
                            [SUB, 2 * BATCH], BF16, tag="slab",
                            name=f"slab{b}_{qp}",
                        )
                        eng = nc.sync if qp % 2 == 0 else nc.scalar
                        eng.dma_start(
                            out=slab[:, :].rearrange("b (a c) -> b a c", a=2),
                            in_=adjt[2 * qp * SUB:(2 * qp + 2) * SUB,
                                     b * BATCH:(b + 1) * BATCH]
                            .rearrange("(a b) c -> b a c", a=2),
                        )
                        for qh in range(2):
                            q = 2 * qp + qh
                            for it in range(NB):
                                nc.tensor.matmul(
                                    acc[it][:, :],
                                    lhsT=slab[:, qh * BATCH + it * SUB:
                                              qh * BATCH + (it + 1) * SUB],
                                    rhs=supbf[q][:, :],
                                    start=(q == 0),
                                    stop=(q == NQ - 1),
                                )
                    for it in range(NB):
                        pt = p_part.tile(
                            [SUB, OUT_F], BF16, tag="pt", name=f"pt{b}_{it}"
                        )
                        nc.vector.tensor_copy(pt[:, :], acc[it][:, :])
                        nc.gpsimd.dma_start(
                            out=rs_in[g][h * BATCH + it * SUB:
                                         h * BATCH + (it + 1) * SUB, :],
                            in_=pt[:, :],
                        )
                    if h == CHUNK_BATCHES[g] - 1:
                        nc.gpsimd.collective_compute(
                            "ReduceScatter",
                            mybir.AluOpType.add,
                            replica_groups=[list(range(C))],
                            ins=[rs_in[g].ap().opt()],
                            outs=[rs_out[g].ap().opt()],
                        )
                        # emit epilogues lagging two chunks behind, so their
                        # RS-completion waits are satisfied on arrival
                        while ep_next <= g - 2:
                            epilogue(ep_next)
                            ep_next += 1
                for g in range(ep_next, G):
                    epilogue(g)

    nc.compile()
    return nc


def _i_perm():
    """Output-row permutation matching the chunked ReduceScatter layout:
    chunk g covers rows [r*R + off_g, r*R + off_g + len_g) for each rank r,
    in rank order, so each chunk's rank-r slice is rank r's own rows."""
    perm = np.empty(N, np.int64)
    pos = 0
    for g in range(G):
        off = CHUNK_START[g] * SUB
        ln = CHUNK_BATCHES[g] * SUB
        for r in range(C):
            base = r * R + off
            perm[pos:pos + ln] = np.arange(base, base + ln)
            pos += ln
    return perm


def _shard_inputs(x, adj, t, weight):
    bf16 = ml_dtypes.bfloat16
    w_bf = np.asarray(weight, np.float32).astype(bf16)
    t0 = float(np.asarray(t, np.float32).reshape(-1)[0])
    tsc = np.empty((128, 2), np.float32)
    tsc[:, 0] = t0
    tsc[:, 1] = 1.0 - t0

    x = np.asarray(x, np.float32)
    adj = np.asarray(adj, np.float32)
    perm = _i_perm()
    adj_p = adj[perm]  # permuted output rows

    in_maps = []
    for c in range(C):
        cols = slice(c * R, (c + 1) * R)
        adjt = np.ascontiguousarray(adj_p[:, cols].T).astype(bf16)  # [R, N]
        xt = np.ascontiguousarray(x[cols].T).astype(bf16)           # [IN_F, R]
        in_maps.append({"adjt": adjt, "xt": xt, "w": w_bf, "tsc": tsc})
    return in_maps


def kernel(x, adj, t, weight):
    if "nc" not in _cached:
        _cached["nc"] = _build()
    nc = _cached["nc"]
    in_maps = _shard_inputs(x, adj, t, weight)
    res = run_bass_kernel_spmd(nc, in_maps, list(range(C)))
    return np.concatenate([res.results[c]["out"] for c in range(C)], axis=0)


# revision 15
# speedup vs baseline: 1.1858x; 1.0698x over previous
"""AdaptiveDiffusionLayer on 8 TRN2 NeuronCores.

out = (1 - t) * support + t * (adj @ support),  support = x @ weight

Strategy (1D row-parallel SpMM + chunked AllGather):
  - Row-shard adj and x across 8 cores (1250 rows each); replicate weight/t.
  - Host-side: pre-transpose + bf16-cast each core's adj shard so the
    contraction index k lands on the SBUF partition axis with unit-stride
    DMA (no on-device transposes). The k rows are permuted to match the
    chunked all-gather layout, and packed as [2, N, 625] so each of the
    two PSUM passes reads contiguous slabs.
  - support_c = x_c @ W (bf16 matmul, fp32 PSUM) is gathered in 5
    pipelined 2MB AllGather chunks (2 of the 10 support subtiles each),
    triggered as soon as each pair of subtiles is bounced to DRAM - the
    SpMM starts right after chunk 0 lands and the AG stream stays ahead
    of PE consumption. All collective-adjacent DMAs (bounce stores,
    gathered-support loads) ride gpsimd's SWDGE semaphore lanes so their
    completion waits are not inflated by the slab-prefetch stream on the
    HWDGE lanes.
  - Main SpMM: 2 passes x 5 PSUM accumulators (125-row i-subtiles) over
    80 uniform 125-row k-tiles; fused epilogue out = t*acc + (1-t)*support_c
    (no collective dependency).
"""

import sys

for _p in ("/opt/trn_rl_repo",):
    if _p not in sys.path:
        sys.path.append(_p)

import numpy as np
import ml_dtypes

from concourse import bass, bacc, mybir, tile
from concourse.bass_utils import run_bass_kernel_spmd

N = 10000
IN_F = 512
OUT_F = 512
C = 8               # cores
R = N // C          # 1250 rows per core
HALF = R // 2       # 625 i-columns per pass
NSUB = 5            # i-subtiles per pass
SUB = HALF // NSUB  # 125 rows per i-subtile / k-tile
G = 5               # all-gather chunks
CH = R // G         # 250 support rows per core per chunk
KT = N // SUB       # 80 k-tiles
QPC = KT // G       # 16 k-tiles per gather chunk

BF16 = mybir.dt.bfloat16
F32 = mybir.dt.float32

_cached = {}


def _build():
    nc = bacc.Bacc("TRN2", target_bir_lowering=False, debug=False, num_devices=C)

    adjt = nc.dram_tensor("adjt", [2, N, HALF], BF16, kind="ExternalInput")
    xt = nc.dram_tensor("xt", [IN_F, R], BF16, kind="ExternalInput")
    w = nc.dram_tensor("w", [IN_F, OUT_F], BF16, kind="ExternalInput")
    tsc = nc.dram_tensor("tsc", [128, 2], F32, kind="ExternalInput")
    out = nc.dram_tensor("out", [R, OUT_F], F32, kind="ExternalOutput")

    sup_in = [nc.dram_tensor(f"sup_in{g}", [CH, OUT_F], BF16) for g in range(G)]
    sup_gath = [
        nc.dram_tensor(f"sup_gath{g}", [C * CH, OUT_F], BF16, addr_space="Shared")
        for g in range(G)
    ]

    NJ = IN_F // 128  # 4 contraction tiles for x @ W

    with tile.TileContext(nc) as tc:
        with (
            tc.tile_pool(name="persist", bufs=1) as p_pers,
            tc.tile_pool(name="supbf_pool", bufs=1) as p_supbf,
            tc.tile_pool(name="sup_pool", bufs=1) as p_sup,
            tc.tile_pool(name="slab_pool", bufs=24) as p_slab,
            tc.tile_pool(name="out_pool", bufs=4) as p_out,
        ):
            xt_sb = p_pers.tile([128, NJ * R], BF16, tag="xt_sb", name="xt_sb")
            w_sb = p_pers.tile([128, NJ * OUT_F], BF16, tag="w_sb", name="w_sb")
            tsc_sb = p_pers.tile([128, 2], F32, tag="tsc_sb", name="tsc_sb")

            for j in range(NJ):
                nc.scalar.dma_start(
                    out=xt_sb[:, j * R:(j + 1) * R],
                    in_=xt[j * 128:(j + 1) * 128, :],
                )
                nc.scalar.dma_start(
                    out=w_sb[:, j * OUT_F:(j + 1) * OUT_F],
                    in_=w[j * 128:(j + 1) * 128, :],
                )
            nc.scalar.dma_start(out=tsc_sb[:, :], in_=tsc[:, :])

            # ---- support_c = x_c @ W; bounce + trigger one AG chunk per
            # pair of 125-row subtiles ----
            supbf = []
            supsc = []
            with tc.tile_pool(name="psum_sup", bufs=3, space="PSUM") as pp_sup:
                for s in range(2 * NSUB):
                    ps = pp_sup.tile([SUB, OUT_F], F32, tag="ps", name=f"ps{s}")
                    for j in range(NJ):
                        nc.tensor.matmul(
                            ps[:, :],
                            lhsT=xt_sb[:, j * R + s * SUB: j * R + (s + 1) * SUB],
                            rhs=w_sb[:, j * OUT_F:(j + 1) * OUT_F],
                            start=(j == 0),
                            stop=(j == NJ - 1),
                        )
                    sb = p_supbf.tile(
                        [SUB, OUT_F], BF16, tag=f"supbf{s}", name=f"supbf{s}"
                    )
                    nc.vector.tensor_copy(sb[:, :], ps[:, :])
                    supbf.append(sb)
                    sc = p_supbf.tile(
                        [SUB, OUT_F], F32, tag=f"supsc{s}", name=f"supsc{s}"
                    )
                    nc.vector.tensor_scalar_mul(
                        sc[:, :], ps[:, :], tsc_sb[0:SUB, 1:2]
                    )
                    supsc.append(sc)
                    g, half = divmod(s, 2)
                    nc.gpsimd.dma_start(
                        out=sup_in[g][half * SUB:(half + 1) * SUB, :],
                        in_=sb[:, :],
                    )
                    if half == 1:
                        nc.gpsimd.collective_compute(
                            "AllGather",
                            mybir.AluOpType.bypass,
                            replica_groups=[list(range(C))],
                            ins=[sup_in[g].ap().opt()],
                            outs=[sup_gath[g].ap().opt()],
                        )

            # ---- load gathered support as [k-part, f] tiles (permuted k),
            # on gpsimd SWDGE lanes so the waits stay tight ----
            sup_tiles = []
            for q in range(KT):
                g, lq = divmod(q, QPC)
                stile = p_sup.tile([SUB, OUT_F], BF16, tag=f"sup{q}", name=f"sup{q}")
                nc.gpsimd.dma_start(
                    out=stile[:, :],
                    in_=sup_gath[g][lq * SUB:(lq + 1) * SUB, :],
                )
                sup_tiles.append(stile)

            # ---- main SpMM: 2 passes x 5 PSUM accumulators over 80 k-tiles ----
            with tc.tile_pool(name="psum_main", bufs=1, space="PSUM") as pp_main:
                for p in range(2):
                    acc = [
                        pp_main.tile(
                            [SUB, OUT_F], F32, tag=f"acc{s}", name=f"acc{p}_{s}"
                        )
                        for s in range(NSUB)
                    ]
                    for q in range(KT):
                        slab = p_slab.tile(
                            [SUB, HALF], BF16, tag="slab", name=f"slab{p}_{q}"
                        )
                        nc.sync.dma_start(
                            out=slab[:, :],
                            in_=adjt[p, q * SUB:(q + 1) * SUB, :],
                        )
                        for s in range(NSUB):
                            nc.tensor.matmul(
                                acc[s][:, :],
                                lhsT=slab[:, s * SUB:(s + 1) * SUB],
                                rhs=sup_tiles[q][:, :],
                                start=(q == 0),
                                stop=(q == KT - 1),
                            )
                    for s in range(NSUB):
                        i_sub = p * NSUB + s
                        ot = p_out.tile(
                            [SUB, OUT_F], F32, tag="ot", name=f"ot{i_sub}"
                        )
                        nc.vector.scalar_tensor_tensor(
                            ot[:, :],
                            acc[s][:, :],
                            tsc_sb[0:SUB, 0:1],
                            supsc[i_sub][:, :],
                            mybir.AluOpType.mult,
                            mybir.AluOpType.add,
                        )
                        nc.scalar.dma_start(
                            out=out[i_sub * SUB:(i_sub + 1) * SUB, :],
                            in_=ot[:, :],
                        )

    nc.compile()
    return nc


def _k_perm():
    """k-row permutation matching the chunked all-gather layout:
    chunk g holds rows [c*R + g*CH, c*R + (g+1)*CH) for each core c,
    concatenated in core order."""
    perm = np.empty(N, np.int64)
    pos = 0
    for g in range(G):
        for c in range(C):
            base = c * R + g * CH
            perm[pos:pos + CH] = np.arange(base, base + CH)
            pos += CH
    return perm


def _shard_inputs(x, adj, t, weight):
    bf16 = ml_dtypes.bfloat16
    w_bf = np.asarray(weight, np.float32).astype(bf16)
    t0 = float(np.asarray(t, np.float32).reshape(-1)[0])
    tsc = np.empty((128, 2), np.float32)
    tsc[:, 0] = t0
    tsc[:, 1] = 1.0 - t0

    x = np.asarray(x, np.float32)
    adj = np.asarray(adj, np.float32)
    perm = _k_perm()

    in_maps = []
    for c in range(C):
        rows = slice(c * R, (c + 1) * R)
        adjT = np.ascontiguousarray(adj[rows].T)[perm].astype(bf16)  # [N, R]
        adjt = np.ascontiguousarray(
            np.stack([adjT[:, :HALF], adjT[:, HALF:]])               # [2, N, 625]
        )
        xt = np.ascontiguousarray(x[rows].T).astype(bf16)            # [IN_F, R]
        in_maps.append({"adjt": adjt, "xt": xt, "w": w_bf, "tsc": tsc})
    return in_maps


def kernel(x, adj, t, weight):
    if "nc" not in _cached:
        _cached["nc"] = _build()
    nc = _cached["nc"]
    in_maps = _shard_inputs(x, adj, t, weight)
    res = run_bass_kernel_spmd(nc, in_maps, list(range(C)))
    return np.concatenate([res.results[c]["out"] for c in range(C)], axis=0)


# revision 16
# speedup vs baseline: 1.2119x; 1.0221x over previous
"""AdaptiveDiffusionLayer on 8 TRN2 NeuronCores.

out = (1 - t) * support + t * (adj @ support),  support = x @ weight

Strategy (1D row-parallel SpMM + chunked AllGather, chunk-major consumption):
  - Row-shard adj and x across 8 cores (1250 rows each); replicate weight/t.
  - Host-side: pre-transpose + bf16-cast each core's adj shard so the
    contraction index k lands on the SBUF partition axis with unit-stride
    DMA (no on-device transposes). The k rows are permuted to match the
    chunked all-gather layout; the i columns are split [1000 | 250] to
    match the 8+2 PSUM-bank visit structure.
  - support_c = x_c @ W (bf16 matmul, fp32 PSUM) is gathered in 5
    pipelined 2MB AllGather chunks, each triggered as soon as its pair of
    support subtiles is bounced to DRAM. Collective-adjacent DMAs (bounce
    stores, gathered-support loads) ride gpsimd's SWDGE semaphore lanes so
    their waits are not inflated by the slab-prefetch stream.
  - Main SpMM consumes each gathered chunk COMPLETELY before the next is
    needed (16 k-tiles x all 10 i-subtiles, visited as 8 banks + 2 banks),
    draining PSUM into persistent SBUF fp32 accumulators after each chunk.
    PE consumption (~44us/chunk) paces the AG delivery (~30us/chunk), so
    total time ~= first-chunk-arrival + total PE work.
  - Fused epilogue: out = t * acc + (1-t) * support_c.
"""

import sys

for _p in ("/opt/trn_rl_repo",):
    if _p not in sys.path:
        sys.path.append(_p)

import numpy as np
import ml_dtypes

from concourse import bass, bacc, mybir, tile
from concourse.bass_utils import run_bass_kernel_spmd

N = 10000
IN_F = 512
OUT_F = 512
C = 8               # cores
R = N // C          # 1250 rows per core
SUB = 125           # i-subtile / k-tile rows
NSUB = R // SUB     # 10 i-subtiles
WA = 8 * SUB        # 1000 i-columns in visit A (8 PSUM banks)
WB = 2 * SUB        # 250 i-columns in visit B (2 PSUM banks)
G = 5               # all-gather chunks
CH = R // G         # 250 support rows per core per chunk
KT = N // SUB       # 80 k-tiles
QPC = KT // G       # 16 k-tiles per gather chunk

BF16 = mybir.dt.bfloat16
F32 = mybir.dt.float32

_cached = {}


def _build():
    nc = bacc.Bacc("TRN2", target_bir_lowering=False, debug=False, num_devices=C)

    adjta = nc.dram_tensor("adjta", [N, WA], BF16, kind="ExternalInput")
    adjtb = nc.dram_tensor("adjtb", [N, WB], BF16, kind="ExternalInput")
    xt = nc.dram_tensor("xt", [IN_F, R], BF16, kind="ExternalInput")
    w = nc.dram_tensor("w", [IN_F, OUT_F], BF16, kind="ExternalInput")
    tsc = nc.dram_tensor("tsc", [128, 2], F32, kind="ExternalInput")
    out = nc.dram_tensor("out", [R, OUT_F], F32, kind="ExternalOutput")

    sup_in = [nc.dram_tensor(f"sup_in{g}", [CH, OUT_F], BF16) for g in range(G)]
    sup_gath = [
        nc.dram_tensor(f"sup_gath{g}", [C * CH, OUT_F], BF16, addr_space="Shared")
        for g in range(G)
    ]

    NJ = IN_F // 128  # 4 contraction tiles for x @ W
    SUPBUF = 32       # ring of gathered-support tiles (2 chunks deep)

    with tile.TileContext(nc) as tc:
        with (
            tc.tile_pool(name="persist", bufs=1) as p_pers,
            tc.tile_pool(name="supbf_pool", bufs=1) as p_supbf,
            tc.tile_pool(name="sup_pool", bufs=1) as p_sup,
            tc.tile_pool(name="slaba_pool", bufs=12) as p_slaba,
            tc.tile_pool(name="slabb_pool", bufs=12) as p_slabb,
            tc.tile_pool(name="accsb_pool", bufs=1) as p_accsb,
            tc.tile_pool(name="out_pool", bufs=4) as p_out,
        ):
            xt_sb = p_pers.tile([128, NJ * R], BF16, tag="xt_sb", name="xt_sb")
            w_sb = p_pers.tile([128, NJ * OUT_F], BF16, tag="w_sb", name="w_sb")
            tsc_sb = p_pers.tile([128, 2], F32, tag="tsc_sb", name="tsc_sb")

            for j in range(NJ):
                nc.scalar.dma_start(
                    out=xt_sb[:, j * R:(j + 1) * R],
                    in_=xt[j * 128:(j + 1) * 128, :],
                )
                nc.scalar.dma_start(
                    out=w_sb[:, j * OUT_F:(j + 1) * OUT_F],
                    in_=w[j * 128:(j + 1) * 128, :],
                )
            nc.scalar.dma_start(out=tsc_sb[:, :], in_=tsc[:, :])

            # ---- support_c = x_c @ W; bounce + trigger one AG chunk per
            # pair of 125-row subtiles ----
            supbf = []
            supsc = []
            with tc.tile_pool(name="psum_sup", bufs=3, space="PSUM") as pp_sup:
                for s in range(NSUB):
                    ps = pp_sup.tile([SUB, OUT_F], F32, tag="ps", name=f"ps{s}")
                    for j in range(NJ):
                        nc.tensor.matmul(
                            ps[:, :],
                            lhsT=xt_sb[:, j * R + s * SUB: j * R + (s + 1) * SUB],
                            rhs=w_sb[:, j * OUT_F:(j + 1) * OUT_F],
                            start=(j == 0),
                            stop=(j == NJ - 1),
                        )
                    sb = p_supbf.tile(
                        [SUB, OUT_F], BF16, tag=f"supbf{s}", name=f"supbf{s}"
                    )
                    nc.vector.tensor_copy(sb[:, :], ps[:, :])
                    supbf.append(sb)
                    sc = p_supbf.tile(
                        [SUB, OUT_F], F32, tag=f"supsc{s}", name=f"supsc{s}"
                    )
                    nc.vector.tensor_scalar_mul(
                        sc[:, :], ps[:, :], tsc_sb[0:SUB, 1:2]
                    )
                    supsc.append(sc)
                    g, half = divmod(s, 2)
                    nc.gpsimd.dma_start(
                        out=sup_in[g][half * SUB:(half + 1) * SUB, :],
                        in_=sb[:, :],
                    )
                    if half == 1:
                        nc.gpsimd.collective_compute(
                            "AllGather",
                            mybir.AluOpType.bypass,
                            replica_groups=[list(range(C))],
                            ins=[sup_in[g].ap().opt()],
                            outs=[sup_gath[g].ap().opt()],
                        )

            # persistent fp32 accumulators for the 10 i-subtiles
            accsb = [
                p_accsb.tile([SUB, OUT_F], F32, tag=f"accsb{i}", name=f"accsb{i}")
                for i in range(NSUB)
            ]

            # ---- main SpMM, chunk-major: for each gathered chunk, run all
            # 10 i-subtiles (8-bank visit + 2-bank visit) over its 16
            # k-tiles, then fold PSUM into the SBUF accumulators ----
            with tc.tile_pool(name="psum_main", bufs=1, space="PSUM") as pp_main:
                for g in range(G):
                    # load this chunk's gathered support tiles (SWDGE lanes)
                    sup_tiles = []
                    for lq in range(QPC):
                        q = g * QPC + lq
                        stile = p_sup.tile(
                            [SUB, OUT_F], BF16,
                            tag=f"sup{q % SUPBUF}", name=f"sup{q}",
                        )
                        nc.gpsimd.dma_start(
                            out=stile[:, :],
                            in_=sup_gath[g][lq * SUB:(lq + 1) * SUB, :],
                        )
                        sup_tiles.append(stile)

                    for visit, (src, width, subs) in enumerate(
                        [(adjta, WA, list(range(8))), (adjtb, WB, [8, 9])]
                    ):
                        nb = len(subs)
                        pa = [
                            pp_main.tile(
                                [SUB, OUT_F], F32,
                                tag=f"pa{i}", name=f"pa{g}_{visit}_{i}",
                            )
                            for i in range(nb)
                        ]
                        slab_pool = p_slaba if visit == 0 else p_slabb
                        for lq in range(QPC):
                            q = g * QPC + lq
                            slab = slab_pool.tile(
                                [SUB, width], BF16,
                                tag="slab", name=f"slab{g}_{visit}_{lq}",
                            )
                            nc.sync.dma_start(
                                out=slab[:, :],
                                in_=src[q * SUB:(q + 1) * SUB, :],
                            )
                            for i in range(nb):
                                nc.tensor.matmul(
                                    pa[i][:, :],
                                    lhsT=slab[:, i * SUB:(i + 1) * SUB],
                                    rhs=sup_tiles[lq][:, :],
                                    start=(lq == 0),
                                    stop=(lq == QPC - 1),
                                )
                        for i in range(nb):
                            i_sub = subs[i]
                            if g == 0:
                                nc.vector.tensor_copy(
                                    accsb[i_sub][:, :], pa[i][:, :]
                                )
                            else:
                                nc.vector.tensor_add(
                                    accsb[i_sub][:, :],
                                    accsb[i_sub][:, :],
                                    pa[i][:, :],
                                )

            # ---- epilogue: out = t * acc + (1-t) * support_c ----
            for i_sub in range(NSUB):
                ot = p_out.tile([SUB, OUT_F], F32, tag="ot", name=f"ot{i_sub}")
                nc.vector.scalar_tensor_tensor(
                    ot[:, :],
                    accsb[i_sub][:, :],
                    tsc_sb[0:SUB, 0:1],
                    supsc[i_sub][:, :],
                    mybir.AluOpType.mult,
                    mybir.AluOpType.add,
                )
                nc.scalar.dma_start(
                    out=out[i_sub * SUB:(i_sub + 1) * SUB, :], in_=ot[:, :]
                )

    nc.compile()
    return nc


def _k_perm():
    """k-row permutation matching the chunked all-gather layout:
    chunk g holds rows [c*R + g*CH, c*R + (g+1)*CH) for each core c,
    concatenated in core order."""
    perm = np.empty(N, np.int64)
    pos = 0
    for g in range(G):
        for c in range(C):
            base = c * R + g * CH
            perm[pos:pos + CH] = np.arange(base, base + CH)
            pos += CH
    return perm


def _shard_inputs(x, adj, t, weight):
    bf16 = ml_dtypes.bfloat16
    w_bf = np.asarray(weight, np.float32).astype(bf16)
    t0 = float(np.asarray(t, np.float32).reshape(-1)[0])
    tsc = np.empty((128, 2), np.float32)
    tsc[:, 0] = t0
    tsc[:, 1] = 1.0 - t0

    x = np.asarray(x, np.float32)
    adj = np.asarray(adj, np.float32)
    perm = _k_perm()

    in_maps = []
    for c in range(C):
        rows = slice(c * R, (c + 1) * R)
        adjT = np.ascontiguousarray(adj[rows].T)[perm].astype(bf16)  # [N, R]
        adjta = np.ascontiguousarray(adjT[:, :WA])                   # [N, 1000]
        adjtb = np.ascontiguousarray(adjT[:, WA:])                   # [N, 250]
        xt = np.ascontiguousarray(x[rows].T).astype(bf16)            # [IN_F, R]
        in_maps.append(
            {"adjta": adjta, "adjtb": adjtb, "xt": xt, "w": w_bf, "tsc": tsc}
        )
    return in_maps


def kernel(x, adj, t, weight):
    if "nc" not in _cached:
        _cached["nc"] = _build()
    nc = _cached["nc"]
    in_maps = _shard_inputs(x, adj, t, weight)
    res = run_bass_kernel_spmd(nc, in_maps, list(range(C)))
    return np.concatenate([res.results[c]["out"] for c in range(C)], axis=0)
